# revision 34
# baseline (speedup 1.0000x reference)
"""Trainium2 Bass kernel for nn_Agent_Memory (LSTM agent forward pass).

Shapes (hardcoded): T=256, B=64, OBS=512, H=1024, A=128, 8 NeuronCores.

This environment has ~100us fixed latency per DMA (serialized per core), so
the design minimizes dma_start count:
  - The shared MLP is replicated (cheap on PE) over all tokens, writing
    hid.T to DRAM in 8 block DMAs.  No collectives anywhere.
  - The scan fuses the w_ih projection: per step the gate PSUM accumulates
    xg (from streamed hid.T blocks, 1 DMA per 8 steps) plus the recurrent
    term.  The done-mask is folded into the per-step transpose identity
    (diag(m) = I * m), so the recurrent h.T state is pre-masked and both
    matmul groups share one accumulation.  Hidden states are staged in SBUF
    and flushed row-major every 8 steps.
  - Heads are sharded by time-block via a partition-id dynamic DMA; logits
    are PE-transposed back for the row-wise softmax; outputs are staged and
    written in single DMAs.
  - All matmuls bf16 (fp32 PSUM); cell state + softmax fp32.
"""

import os

import numpy as np
import ml_dtypes

import concourse.bass as bass
from concourse import bacc
import concourse.mybir as mybir
import concourse.tile as tile
from concourse.bass import ts, ds
from concourse.masks import make_identity

T, B, OBS, H, A = 256, 64, 512, 1024, 128
G4 = 4 * H
NCORES = 8
F32 = mybir.dt.float32
BF16 = mybir.dt.bfloat16
AF = mybir.ActivationFunctionType
OP = mybir.AluOpType
BF = ml_dtypes.bfloat16

HC = H // 128    # 8 K-chunks of H
PC = 4           # pointwise chunks per step
PW = H // PC     # 256 h-dims per pointwise chunk
GW = 4 * PW      # 1024 gate cols per pointwise chunk
SBLK = 8         # scan steps per hid.T/hs flush block


def _gate_perm():
    """4H gate rows: torch blocks [i,f,g,o] -> per pointwise-chunk layout
    [i_n, f_n, o_n, g_n] (PW wide each)."""
    idx = []
    for n in range(PC):
        for blk in (0, 1, 3, 2):  # i, f, o, g
            base = blk * H + n * PW
            idx.extend(range(base, base + PW))
    return np.array(idx, dtype=np.int64)


def _load_wT(nc, dst, src, kn, fn):
    nc.sync.dma_start(
        dst[:].rearrange("p (k f) -> p k f", k=kn),
        src[:].rearrange("(k p) f -> p k f", p=128))


def build_program(t_total=T, n_cores=NCORES, phase=3):
    TBLK = t_total // n_cores
    TOK = TBLK * B
    NTT = TOK // 128
    NCH = TOK // 512
    TBK = min(2048, t_total * B)  # phase-1 token block
    NB1 = (t_total * B) // TBK
    assert TOK % 512 == 0 and (t_total * B) % TBK == 0
    nsteps = int(os.environ.get("DBG_STEPS", "0")) or t_total

    nc = bacc.Bacc(None, num_devices=n_cores)

    # ---------------- I/O ----------------
    xT_in = nc.declare_dram_parameter("xT_in", [OBS, t_total * B], BF16,
                                      isOutput=False)
    covT_sh = nc.declare_dram_parameter("covT_sh", [A, TOK], BF16,
                                        isOutput=False)
    act_sh = nc.declare_dram_parameter("act_sh", [TOK], F32, isOutput=False)
    done_all = nc.declare_dram_parameter("done_all", [t_total * B], F32,
                                         isOutput=False)
    h0 = nc.declare_dram_parameter("h0", [B, H], F32, isOutput=False)
    c0 = nc.declare_dram_parameter("c0", [B, H], F32, isOutput=False)

    wp = {}
    for name, shape in [
        ("snw1T", [OBS, H]), ("snw2T", [H, H]),
        ("wihT", [H, G4]), ("whhT", [H, G4]),
        ("acw1T", [H, H]), ("cww1T", [H, H]), ("crw1T", [H, H]),
        ("covw1T", [A, H]),
        ("acw2T", [H, A]), ("cww2T", [H, A]), ("covw2T", [H, A]),
        ("crw2T", [H, 1]),
    ]:
        wp[name] = nc.declare_dram_parameter(name, shape, BF16,
                                             isOutput=False)
    bblob = nc.declare_dram_parameter("bblob", [128, 52], F32,
                                      isOutput=False)
    bgB = nc.declare_dram_parameter("bgB", [128, G4], BF16, isOutput=False)

    probs_o = nc.declare_dram_parameter("probs_o", [TOK, A], F32,
                                        isOutput=True)
    lpa_o = nc.declare_dram_parameter("lpa_o", [TOK], F32, isOutput=True)
    ent_o = nc.declare_dram_parameter("ent_o", [TOK], F32, isOutput=True)
    val_o = nc.declare_dram_parameter("val_o", [TOK], F32, isOutput=True)
    hT_o = nc.declare_dram_parameter("hT_o", [B, H], F32, isOutput=True)
    cT_o = nc.declare_dram_parameter("cT_o", [B, H], F32, isOutput=True)

    with tile.TileContext(nc) as tc:
        with tc.tile_pool(name="dram", bufs=1, space="DRAM") as dram, \
             tc.tile_pool(name="cst", bufs=1) as cst:
            # hid.T blocked: [blk, 128, kc, TBK]
            hidT_d = dram.tile([NB1, 128, HC, TBK], BF16, tag="hidT_d")
            # hidden states row-major: [t, B, H]
            hsR = dram.tile([t_total, B, H], BF16, tag="hsR")

            I64b = cst.tile([128, 64], BF16, tag="I64b")
            nc.gpsimd.memset(I64b[:], 0.0)
            nc.gpsimd.affine_select(
                out=I64b[0:64, :], in_=I64b[0:64, :],
                compare_op=OP.not_equal, fill=1.0, base=0,
                pattern=[[-1, 64]], channel_multiplier=1)
            nc.gpsimd.affine_select(
                out=I64b[ds(64, 64), :], in_=I64b[ds(64, 64), :],
                compare_op=OP.not_equal, fill=1.0, base=-64,
                pattern=[[-1, 64]], channel_multiplier=1)
            I128b = cst.tile([128, 128], BF16, tag="I128b")
            make_identity(nc, I128b[:])

            m_sb = cst.tile([128, t_total], F32, tag="m_sb")
            nc.sync.dma_start(m_sb[0:B, :],
                              done_all[:].rearrange("(t b) -> b t", b=B))
            nc.sync.dma_start(m_sb[ds(B, B), :],
                              done_all[:].rearrange("(t b) -> b t", b=B))
            nc.vector.tensor_scalar(m_sb[:], m_sb[:], -1.0, 1.0,
                                    OP.mult, OP.add)

            ball = cst.tile([128, 52], F32, tag="ball")
            nc.sync.dma_start(ball[:], bblob[:])
            bias_sb = {}
            for i, name in enumerate(
                    ("snb1", "snb2", "acb1", "cwb1", "crb1", "covb1")):
                bias_sb[name] = ball[:, ds(i * HC, HC)]
            for i, name in enumerate(("acb2", "cwb2", "covb2")):
                bias_sb[name] = ball[:, ds(48 + i, 1)]
            bias_sb["crb2"] = ball[0:1, ds(51, 1)]
            bg1 = cst.tile([1, G4], BF16, tag="bg1")
            nc.sync.dma_start(bg1[:], bgB[0:1, :])
            ones1 = cst.tile([1, 64], BF16, tag="ones1")
            nc.gpsimd.memset(ones1[:], 1.0)

            # ============ Phase 1: shared MLP (replicated, blocked) ========
            with tc.tile_pool(name="ffw", bufs=1) as ffw, \
                 tc.tile_pool(name="ffa", bufs=2) as ffa, \
                 tc.tile_pool(name="psA", bufs=6, space="PSUM") as psA:

                snw1_sb = ffw.tile([128, (OBS // 128) * H], BF16, tag="snw1")
                _load_wT(nc, snw1_sb, wp["snw1T"], OBS // 128, H)
                snw2_sb = ffw.tile([128, HC * H], BF16, tag="snw2")
                _load_wT(nc, snw2_sb, wp["snw2T"], HC, H)

                for blk in range(NB1):
                    xTb = ffa.tile([128, (OBS // 128) * TBK], BF16,
                                   tag="xTb")
                    nc.sync.dma_start(
                        xTb[:].rearrange("p (k f) -> p k f", k=OBS // 128),
                        xT_in[:, ds(blk * TBK, TBK)].rearrange(
                            "(k p) f -> p k f", p=128))
                    h1T = ffa.tile([128, HC * TBK], BF16, tag="h1T")
                    for m in range(HC):
                        for nch in range(TBK // 512):
                            ps = psA.tile([128, 512], F32, tag="ps")
                            for kc in range(OBS // 128):
                                nc.tensor.matmul(
                                    ps[:],
                                    snw1_sb[:, ds(kc * H + m * 128, 128)],
                                    xTb[:, ds(kc * TBK + nch * 512, 512)],
                                    start=(kc == 0),
                                    stop=(kc == OBS // 128 - 1))
                            nc.scalar.activation(
                                h1T[:, ds(m * TBK + nch * 512, 512)], ps[:],
                                AF.Tanh, bias=bias_sb["snb1"][:, ds(m, 1)])
                    h2T = ffa.tile([128, HC * TBK], BF16, tag="h2T")
                    for m in range(HC):
                        for nch in range(TBK // 512):
                            ps = psA.tile([128, 512], F32, tag="ps")
                            for kc in range(HC):
                                nc.tensor.matmul(
                                    ps[:],
                                    snw2_sb[:, ds(kc * H + m * 128, 128)],
                                    h1T[:, ds(kc * TBK + nch * 512, 512)],
                                    start=(kc == 0), stop=(kc == HC - 1))
                            nc.scalar.activation(
                                h2T[:, ds(m * TBK + nch * 512, 512)], ps[:],
                                AF.Tanh, bias=bias_sb["snb2"][:, ds(m, 1)])
                    nc.sync.dma_start(
                        hidT_d[blk],
                        h2T[:].rearrange("p (k f) -> p k f", k=HC))

            # ============ Phase 2/3: LSTM scan (replicated, fused xg) ======
            if phase >= 2:
                _emit_scan(nc, tc, t_total, nsteps, TBK, wp, hidT_d, hsR,
                           m_sb, I64b, h0, c0, hT_o, cT_o, bg1, ones1)

            # ============ Phase 4: heads ============
            if phase >= 3:
                _emit_heads(nc, tc, t_total, n_cores, TOK, NTT, NCH, wp,
                            bias_sb, hsR, covT_sh, I128b, act_sh,
                            probs_o, lpa_o, ent_o, val_o)

    nc.compile()
    return nc


def _emit_scan(nc, tc, t_total, nsteps, TBK, wp, hidT_d, hsR, m_sb,
               I64b, h0, c0, hT_o, cT_o, bg1, ones1):
    NWIN = TBK // (SBLK * B)  # hid.T windows per phase-1 block
    with tc.tile_pool(name="scw", bufs=1) as scw, \
         tc.tile_pool(name="scst", bufs=1) as scst, \
         tc.tile_pool(name="schid", bufs=2) as schid, \
         tc.tile_pool(name="schs", bufs=2) as schs, \
         tc.tile_pool(name="scsm", bufs=2) as scsm, \
         tc.tile_pool(name="psU", bufs=4, space="PSUM") as psU, \
         tc.tile_pool(name="psHT", bufs=2, space="PSUM") as psHT:

        whh_sb = scw.tile([128, HC * G4], BF16, tag="whh")
        _load_wT(nc, whh_sb, wp["whhT"], HC, G4)
        wih_sb = scw.tile([128, HC * G4], BF16, tag="wih")
        _load_wT(nc, wih_sb, wp["wihT"], HC, G4)

        c_sb = scst.tile([B, H], F32, tag="c_sb")
        nc.sync.dma_start(c_sb[:], c0[:])
        h0f = scst.tile([B, H], F32, tag="hf", name="h0f")
        nc.sync.dma_start(h0f[:], h0[:])
        h0b = schs.tile([B, SBLK * H], BF16, tag="hss", name="h0b")
        nc.scalar.copy(h0b[0:B, 0:H], h0f[:])
        hT_bufs = [scst.tile([128, HC * B], BF16, tag="hT_sb0",
                             name="hT_sb0"),
                   scst.tile([128, HC * B], BF16, tag="hT_sb1",
                             name="hT_sb1")]
        diag = scsm.tile([128, 64], BF16, tag="diag")
        nc.vector.tensor_scalar(diag[:], I64b[:], m_sb[:, ds(0, 1)], None,
                                OP.mult)
        for n in range(HC):
            pht = psHT.tile([128, B], F32, tag="pht")
            nc.tensor.matmul(pht[:], h0b[0:B, ts(n, 128)], diag[0:64, :],
                             start=True, stop=True)
            nc.scalar.copy(hT_bufs[0][:, ts(n, B)], pht[:])

        hs_stage = None
        for t in range(nsteps):
            win = t // SBLK
            si = t % SBLK
            if si == 0:
                hidT_b = schid.tile([128, HC * SBLK * B], BF16, tag="hidb")
                nc.sync.dma_start(
                    hidT_b[:].rearrange("p (k f) -> p k f", k=HC),
                    hidT_d[win // NWIN, :, :,
                           ds((win % NWIN) * SBLK * B, SBLK * B)])
                hs_stage = schs.tile([B, SBLK * H], BF16, tag="hss")
            hT_cur = hT_bufs[t % 2]
            hT_nxt = hT_bufs[(t + 1) % 2]
            mt = m_sb[0:B, ds(t, 1)]
            hrow = hs_stage[:, ds(si * H, H)]
            for n in range(PC):
                pus = []
                for half in range(2):
                    puf = psU.tile([128, 512], F32, tag="u",
                                   name=f"pu{t}_{n}_{half}")
                    pu = puf[0:B, :]
                    pus.append(pu)
                    gc0 = n * GW + half * 512
                    nc.tensor.matmul(pu[:], ones1[:], bg1[:, ds(gc0, 512)],
                                     start=True, stop=False)
                    for j in range(HC):
                        nc.tensor.matmul(
                            pu[:],
                            hidT_b[:, ds(j * SBLK * B + si * B, B)],
                            wih_sb[:, ds(j * G4 + gc0, 512)],
                            start=False, stop=False)
                    for j in range(HC):
                        nc.tensor.matmul(
                            pu[:],
                            hT_cur[:, ts(j, B)],
                            whh_sb[:, ds(j * G4 + gc0, 512)],
                            start=False, stop=(j == HC - 1))
                sif = scsm.tile([B, 512], F32, tag="sif")
                nc.scalar.activation(sif[:], pus[0][:], AF.Sigmoid)
                so = scsm.tile([B, PW], BF16, tag="so")
                nc.scalar.activation(so[:], pus[1][:, 0:PW], AF.Sigmoid)
                tg = scsm.tile([B, PW], F32, tag="tg")
                nc.scalar.activation(tg[:], pus[1][:, ds(PW, PW)], AF.Tanh)
                t1 = scsm.tile([B, PW], BF16, tag="t1")
                nc.vector.tensor_tensor(t1[:], sif[:, 0:PW], tg[:], OP.mult)
                t2 = scsm.tile([B, PW], F32, tag="t2")
                nc.vector.scalar_tensor_tensor(
                    t2[:], sif[:, ds(PW, PW)], mt, c_sb[:, ts(n, PW)],
                    OP.mult, OP.mult)
                nc.vector.tensor_tensor(c_sb[:, ts(n, PW)], t1[:], t2[:],
                                        OP.add)
                th = scsm.tile([B, PW], BF16, tag="th")
                nc.scalar.activation(th[:], c_sb[:, ts(n, PW)], AF.Tanh)
                nc.vector.tensor_tensor(hrow[:, ts(n, PW)],
                                        so[:], th[:], OP.mult)
            # masked transposes for the next step's recurrent state
            if t + 1 < nsteps:
                diag = scsm.tile([128, 64], BF16, tag="diag")
                nc.vector.tensor_scalar(diag[:], I64b[:],
                                        m_sb[:, ds(t + 1, 1)], None,
                                        OP.mult)
                dsl = diag[0:64, :]
                for hn in range(HC):
                    pht = psHT.tile([128, B], F32, tag="pht")
                    nc.tensor.matmul(pht[:], hrow[:, ts(hn, 128)], dsl,
                                     start=True, stop=True)
                    nc.scalar.copy(hT_nxt[:, ts(hn, B)], pht[:])
            if si == SBLK - 1:
                nc.sync.dma_start(
                    hsR[ds(win * SBLK, SBLK)].rearrange("t b h -> b t h"),
                    hs_stage[:].rearrange("b (t h) -> b t h", t=SBLK))

        hTf = scst.tile([B, H], F32, tag="hf", name="hTf")
        sil = (nsteps - 1) % SBLK
        nc.scalar.copy(hTf[:], hs_stage[:, ds(sil * H, H)])
        nc.sync.dma_start(hT_o[:], hTf[:])
        nc.sync.dma_start(cT_o[:], c_sb[:])


def _emit_heads(nc, tc, t_total, n_cores, TOK, NTT, NCH, wp, bias_sb, hsR,
                covT_sh, I128b, act_sh,
                probs_o, lpa_o, ent_o, val_o):
    with tc.tile_pool(name="hda", bufs=1) as hda, \
         tc.tile_pool(name="hdw", bufs=1) as hdw, \
         tc.tile_pool(name="hsm", bufs=3) as hsm, \
         tc.tile_pool(name="psH", bufs=3, space="PSUM") as psH, \
         tc.tile_pool(name="psV", bufs=2, space="PSUM") as psV:

        actc_sb = hda.tile([128, NTT], F32, tag="actc")
        nc.sync.dma_start(actc_sb[:],
                          act_sh[:].rearrange("(n p) -> p n", p=128))
        iota_sb = hda.tile([128, 128], F32, tag="iota")
        nc.gpsimd.iota(iota_sb[:], pattern=[[1, 128]], base=0,
                       channel_multiplier=0,
                       allow_small_or_imprecise_dtypes=True)
        # my token block of hidden states, row-major, then transpose on PE
        hrows = hda.tile([128, NTT * H], BF16, tag="hrows")
        if n_cores > 1:
            pid = nc.sync.partition_id()
            hsR_v = hsR[:].rearrange("(c n two) b h -> c n (two b) h",
                                     c=n_cores, two=2)
            nc.sync.dma_start(
                hrows[:].rearrange("p (n h) -> p n h", n=NTT),
                hsR_v[bass.ds(pid, 1), :, :, :].rearrange(
                    "c n p h -> p (c n) h"))
        else:
            hsR_v = hsR[:].rearrange("(n two) b h -> n (two b) h",
                                     two=2)
            nc.sync.dma_start(
                hrows[:].rearrange("p (n h) -> p n h", n=NTT),
                hsR_v.rearrange("n p h -> p n h"))

        hidT = hda.tile([128, HC * TOK], BF16, tag="hidT")
        for tt in range(NTT):
            for hc in range(HC):
                pst = psV.tile([128, 128], BF16, tag="pstc")
                nc.tensor.transpose(
                    pst[:], hrows[:, ds(tt * H + hc * 128, 128)], I128b[:])
                nc.scalar.copy(
                    hidT[:, ds(hc * TOK + tt * 128, 128)], pst[:])

        covT = hda.tile([128, TOK], BF16, tag="covT")
        nc.sync.dma_start(covT[:], covT_sh[:])

        def layer1(wname, bname, rhs, rhs_tok, kc_n):
            w_sb = hdw.tile([128, kc_n * H], BF16, tag="hw1",
                            name=f"w1_{wname}")
            _load_wT(nc, w_sb, wp[wname], kc_n, H)
            a1 = hda.tile([128, HC * TOK], BF16, tag="a1T",
                          name=f"a1_{wname}")
            for m in range(HC):
                for nch in range(NCH):
                    ps = psH.tile([128, 512], F32, tag="psh",
                                  name=f"ps1_{wname}_{m}_{nch}")
                    for kc in range(kc_n):
                        nc.tensor.matmul(
                            ps[:],
                            w_sb[:, ds(kc * H + m * 128, 128)],
                            rhs[:, ds(kc * rhs_tok + nch * 512, 512)],
                            start=(kc == 0), stop=(kc == kc_n - 1))
                    nc.scalar.activation(
                        a1[:, ds(m * TOK + nch * 512, 512)], ps[:],
                        AF.Tanh, bias=bias_sb[bname][:, ds(m, 1)])
            return a1

        def layer2(wname, bname, a1, out_f32):
            w_sb = hdw.tile([128, HC * A], BF16, tag="hw2",
                            name=f"w2_{wname}")
            _load_wT(nc, w_sb, wp[wname], HC, A)
            for nch in range(NCH):
                ps = psH.tile([128, 512], F32, tag="psh",
                              name=f"ps2_{wname}_{nch}")
                for kc in range(HC):
                    nc.tensor.matmul(
                        ps[:],
                        w_sb[:, ds(kc * A, A)],
                        a1[:, ds(kc * TOK + nch * 512, 512)],
                        start=(kc == 0), stop=(kc == HC - 1))
                nc.scalar.activation(
                    out_f32[:, ts(nch, 512)], ps[:], AF.Identity,
                    bias=bias_sb[bname][:, ds(0, 1)])

        logitsT = hda.tile([128, TOK], F32, tag="logitsT")
        cwl = hda.tile([128, TOK], F32, tag="cwl")
        covl = hda.tile([128, TOK], F32, tag="covl")

        a1 = layer1("acw1T", "acb1", hidT, TOK, HC)
        layer2("acw2T", "acb2", a1, logitsT)
        a1 = layer1("cww1T", "cwb1", hidT, TOK, HC)
        layer2("cww2T", "cwb2", a1, cwl)
        a1 = layer1("covw1T", "covb1", covT, TOK, 1)
        layer2("covw2T", "covb2", a1, covl)

        nc.vector.tensor_tensor(cwl[:], cwl[:], covl[:], OP.mult)
        nc.vector.tensor_tensor(logitsT[:], logitsT[:], cwl[:], OP.add)
        logitsB = hda.tile([128, TOK], BF16, tag="logitsB")
        nc.vector.tensor_copy(logitsB[:], logitsT[:])

        # critic
        a1 = layer1("crw1T", "crb1", hidT, TOK, HC)
        crw2_sb = hdw.tile([128, HC], BF16, tag="crw2")
        _load_wT(nc, crw2_sb, wp["crw2T"], HC, 1)
        val_sb = hda.tile([1, TOK], F32, tag="val_sb")
        for nch in range(NCH):
            ps = psV.tile([1, 512], F32, tag="psv", name=f"psv{nch}")
            for kc in range(HC):
                nc.tensor.matmul(
                    ps[:],
                    crw2_sb[:, ds(kc, 1)],
                    a1[:, ds(kc * TOK + nch * 512, 512)],
                    start=(kc == 0), stop=(kc == HC - 1))
            nc.scalar.activation(val_sb[:, ts(nch, 512)], ps[:],
                                 AF.Identity,
                                 bias=bias_sb["crb2"][:, ds(0, 1)])
        nc.sync.dma_start(val_o[:], val_sb[:])

        # softmax / entropy / logp(action); outputs staged, single DMAs
        prs = hda.tile([128, NTT * A], F32, tag="prs")
        ent_c = hda.tile([128, NTT], F32, tag="ent_c")
        lpa_c = hda.tile([128, NTT], F32, tag="lpa_c")
        for tt in range(NTT):
            psl = psV.tile([128, 128], BF16, tag="pstc", name=f"psl{tt}")
            nc.tensor.transpose(psl[:], logitsB[:, ts(tt, 128)], I128b[:])
            lg = psl[:]
            e = hsm.tile([128, 128], F32, tag="e")
            ssum = hsm.tile([128, 1], F32, tag="ssum")
            nc.scalar.activation(e[:], lg, AF.Exp, accum_out=ssum[:])
            r = hsm.tile([128, 1], F32, tag="r")
            nc.vector.reciprocal(r[:], ssum[:])
            pr = prs[:, ts(tt, A)]
            nc.vector.tensor_scalar(pr[:], e[:], r[:], None, OP.mult)
            ls = hsm.tile([128, 1], F32, tag="ls")
            nc.scalar.activation(ls[:], ssum[:], AF.Ln)
            pl = hsm.tile([128, 1], F32, tag="pl")
            junk = hsm.tile([128, 128], F32, tag="junk")
            nc.vector.tensor_tensor(junk[:], pr[:], lg, OP.mult)
            nc.vector.tensor_reduce(pl[:], junk[:], mybir.AxisListType.X,
                                    OP.add)
            nc.vector.tensor_sub(ent_c[:, ds(tt, 1)], ls[:], pl[:])
            oh = hsm.tile([128, 128], F32, tag="oh")
            nc.vector.tensor_scalar(oh[:], iota_sb[:],
                                    actc_sb[:, ds(tt, 1)], None,
                                    OP.is_equal)
            la = hsm.tile([128, 1], F32, tag="la")
            nc.vector.tensor_tensor(junk[:], oh[:], lg, OP.mult)
            nc.vector.tensor_reduce(la[:], junk[:], mybir.AxisListType.X,
                                    OP.add)
            nc.vector.tensor_sub(lpa_c[:, ds(tt, 1)], la[:], ls[:])
        nc.sync.dma_start(
            probs_o[:].rearrange("(n p) a -> p n a", p=128),
            prs[:].rearrange("p (n a) -> p n a", n=NTT))
        nc.sync.dma_start(ent_o[:].rearrange("(n p) -> p n", p=128),
                          ent_c[:])
        nc.sync.dma_start(lpa_o[:].rearrange("(n p) -> p n", p=128),
                          lpa_c[:])


# ---------------------------------------------------------------------------
# Host glue
# ---------------------------------------------------------------------------
_CACHE = {}


def _get_program(t_total, n_cores, phase=3):
    key = (t_total, n_cores, phase, os.environ.get("DBG_STEPS", ""))
    if key not in _CACHE:
        _CACHE[key] = build_program(t_total, n_cores, phase)
    return _CACHE[key]



def _bias_blob(inputs):
    f32 = np.float32
    bb = np.zeros((128, 52), f32)
    for i, k in enumerate(("sn_b1", "sn_b2", "ac_b1", "cw_b1", "cr_b1",
                           "cov_b1")):
        bb[:, i * 8:(i + 1) * 8] = np.asarray(
            inputs[k], f32).reshape(8, 128).T
    for i, k in enumerate(("ac_b2", "cw_b2", "cov_b2")):
        bb[:, 48 + i] = np.asarray(inputs[k], f32)
    bb[0, 51] = np.asarray(inputs["cr_b2"], f32)[0]
    return bb


def prep_in_maps(inputs, t_total=T, n_cores=NCORES):
    TBLK = t_total // n_cores
    TOK = TBLK * B
    perm = _gate_perm()
    f32 = np.float32

    def bf(a):
        return np.ascontiguousarray(np.asarray(a, dtype=f32)).astype(BF)

    x = np.asarray(inputs["x"], dtype=f32)[:t_total * B]
    com = {
        "xT_in": bf(x.T),
        "done_all": np.ascontiguousarray(
            np.asarray(inputs["done"])[:t_total * B], dtype=f32),
        "h0": np.ascontiguousarray(inputs["h0"][0], dtype=f32),
        "c0": np.ascontiguousarray(inputs["c0"][0], dtype=f32),
        "snw1T": bf(np.asarray(inputs["sn_w1"], dtype=f32).T),
        "snw2T": bf(np.asarray(inputs["sn_w2"], dtype=f32).T),
        "wihT": bf(np.asarray(inputs["w_ih"], dtype=f32)[perm].T),
        "whhT": bf(np.asarray(inputs["w_hh"], dtype=f32)[perm].T),
        "acw1T": bf(np.asarray(inputs["ac_w1"], dtype=f32).T),
        "cww1T": bf(np.asarray(inputs["cw_w1"], dtype=f32).T),
        "crw1T": bf(np.asarray(inputs["cr_w1"], dtype=f32).T),
        "covw1T": bf(np.asarray(inputs["cov_w1"], dtype=f32).T),
        "acw2T": bf(np.asarray(inputs["ac_w2"], dtype=f32).T),
        "cww2T": bf(np.asarray(inputs["cw_w2"], dtype=f32).T),
        "covw2T": bf(np.asarray(inputs["cov_w2"], dtype=f32).T),
        "crw2T": bf(np.asarray(inputs["cr_w2"], dtype=f32).T),
        "bblob": _bias_blob(inputs),
        "bgB": np.ascontiguousarray(np.broadcast_to(
            (np.asarray(inputs["b_ih"], dtype=f32)
             + np.asarray(inputs["b_hh"], dtype=f32))[perm],
            (128, G4))).astype(BF),
    }
    cov = np.asarray(inputs["coverage_hist"], dtype=f32)
    act = np.asarray(inputs["action"])
    in_maps = []
    for k in range(n_cores):
        sl = slice(k * TOK, (k + 1) * TOK)
        m = dict(com)
        m["covT_sh"] = bf(cov[sl].T)
        m["act_sh"] = np.ascontiguousarray(act[sl], dtype=f32)
        in_maps.append(m)
    return in_maps


def assemble(results, t_total=T, n_cores=NCORES):
    probs = np.concatenate([r["probs_o"] for r in results], axis=0)
    lpa = np.concatenate([r["lpa_o"] for r in results], axis=0)
    ent = np.concatenate([r["ent_o"] for r in results], axis=0)
    val = np.concatenate([r["val_o"] for r in results], axis=0)[:, None]
    hT = results[0]["hT_o"][None]
    cT = results[0]["cT_o"][None]
    return (np.ascontiguousarray(probs, np.float32),
            np.ascontiguousarray(lpa, np.float32),
            np.ascontiguousarray(ent, np.float32),
            np.ascontiguousarray(val, np.float32),
            np.ascontiguousarray(hT, np.float32),
            np.ascontiguousarray(cT, np.float32))


def run(inputs, t_total=T, n_cores=NCORES, phase=3, **kw):
    from concourse.bass_utils import run_bass_kernel_spmd
    nc = _get_program(t_total, n_cores, phase)
    in_maps = prep_in_maps(inputs, t_total, n_cores)
    br = run_bass_kernel_spmd(nc, in_maps, list(range(n_cores)), **kw)
    return assemble(br.results, t_total, n_cores), br


def kernel(**inputs):
    out, _ = run(inputs)
    return out
